# revision 13
# baseline (speedup 1.0000x reference)
"""Non-local block (no softmax) on 8 Trainium2 cores, data-parallel over batch.

Math: per sample X [N=4096, C=256] (N = 64*64 spatial, C channels):
    theta = X Wt, phi = X Wp, g = X Wg          (biases are zero)
    y = (theta phi^T / N) g  ->  associativity (no softmax):
      y = X Wt A / N,  A = Wp^T G Wg,  G = X^T X
    z = y (Ww * s) + t2 + X  =  X (M2 + I) + t2
      M2 = Wt (A/N) (Ww * s),  s = gamma*rsqrt(var+eps),
      t2 = (b_W - mean)*s + beta
so per core: G = X^T X, a small 256x256 chain to build M2' = M2 + I,
then z^T = M2'^T X^T + t2. One sample per NeuronCore.

v2 layout notes:
  - x rows are loaded permuted: partition p holds rows 4p..4p+3 of each
    512-row block (2KB contiguous per partition per DMA -> efficient
    descriptors). G is row-permutation invariant.
  - X^T windows carry the permuted order f = j*128 + p  <->  n = 4p + j;
    the phase-3 PSUM drain un-permutes via a rearranged read AP so the
    z stores are contiguous.
  - G symmetry: G[128:,0:128] is rebuilt as transpose(G[0:128,128:]).
  - chain runs in bf16; z output is stored bf16 and upcast on host.
"""

import numpy as np
import ml_dtypes

B, H, W, C = 8, 64, 64, 256
IC = C // 2
N = H * W
BN_EPS = 1e-3

_CACHE = {}


def _build_nc():
    import concourse.bacc as bacc
    import concourse.mybir as mybir
    import concourse.tile as tile

    F32 = mybir.dt.float32
    BF16 = mybir.dt.bfloat16

    nc = bacc.Bacc("TRN2", target_bir_lowering=False, debug=False)

    x_d = nc.dram_tensor("x", [N, C], BF16, kind="ExternalInput")
    lt_d = nc.dram_tensor("lt", [128, 512], BF16, kind="ExternalInput")
    rp_d = nc.dram_tensor("rp", [128, 512], BF16, kind="ExternalInput")
    idb_d = nc.dram_tensor("idb", [128, 128], BF16, kind="ExternalInput")
    t2c_d = nc.dram_tensor("t2c", [128, 2], F32, kind="ExternalInput")
    z_d = nc.dram_tensor("zt", [C, N], BF16, kind="ExternalOutput")

    with tile.TileContext(nc) as tc:
        with (
            tc.tile_pool(name="const", bufs=1) as cpool,
            tc.tile_pool(name="big", bufs=1) as bpool,
            tc.tile_pool(name="zs", bufs=5) as zpool,
            tc.tile_pool(name="psg", bufs=1, space="PSUM") as psg,
            tc.tile_pool(name="psw", bufs=6, space="PSUM") as psw,
        ):
            lt = cpool.tile([128, 512], BF16, tag="lt")
            rp = cpool.tile([128, 512], BF16, tag="rp")
            id2 = cpool.tile([128, 512], BF16, tag="id2")
            idb = cpool.tile([128, 128], BF16, tag="idb")
            t2c = cpool.tile([128, 2], F32, tag="t2c")
            idz = cpool.tile([128, 512], BF16, tag="idz")
            nc.vector.memset(idz[:], 0.0)
            # aux tensors ride the gpsimd (SWDGE) ring so the two HWDGE
            # rings carry nothing but x tiles / z stores
            nc.gpsimd.dma_start(idb[:], idb_d[:])
            nc.gpsimd.dma_start(t2c[:], t2c_d[:])
            # id2 = [eye128 | 0 ; 0 | eye128] assembled from idb on-chip
            nc.vector.memset(id2[:], 0.0)
            nc.vector.tensor_copy(id2[:, 0:128], idb[:])
            nc.scalar.copy(id2[:, 384:512], idb[:])

            xnat_t = [bpool.tile([128, 1024], BF16, tag=f"x_nat{u}", name=f"x_nat{u}")
                      for u in range(8)]
            # X^T halves: xt[k][c', u*512 + j*128 + p] = X[u*512 + 4p + j, 128k + c']
            xt_t = [bpool.tile([128, 4096], BF16, tag=f"xt{k}", name=f"xt{k}")
                    for k in range(2)]

            # PE warmup on zeros: keep the clock ramping while x loads
            wu = psw.tile([128, 512], F32, tag="w")
            for _ in range(12):
                nc.tensor.matmul(wu[:, 0:128], idz[:, 0:128], idz[:, 0:128],
                                 start=True, stop=True, skip_group_check=True)

            # ---- phase 1: load x; G = X^T X (sym-reduced); X^T via PE transpose
            # u0 arrives as two half-tiles so G can start ~2us earlier
            for h in range(2):
                nc.sync.dma_start(
                    xnat_t[0][:, h * 512:(h + 1) * 512].rearrange(
                        "p (j c) -> p j c", j=2),
                    x_d[0:512, :].rearrange("(p jj j) c -> p jj j c",
                                            p=128, jj=2, j=2)[:, h, :, :])
            for u in range(1, 8):
                # partition p <- rows 4p..4p+3 (2KB contiguous per partition)
                ring = (nc.gpsimd if u == 5
                        else nc.sync if u % 2 == 0 else nc.scalar)
                ring.dma_start(
                    xnat_t[u][:].rearrange("p (j c) -> p j c", j=4),
                    x_d[u * 512:(u + 1) * 512, :].rearrange(
                        "(p j) c -> p j c", p=128))

            g0 = psg.tile([128, 256], F32, tag="g0")
            g1h = psg.tile([128, 128], F32, tag="g1h")
            for u in range(8):
                if u == 2:
                    nc.gpsimd.dma_start(lt[:], lt_d[:])
                    nc.gpsimd.dma_start(rp[:], rp_d[:])
                xn = xnat_t[u]
                tpb = [psw.tile([128, 512], BF16, tag="w", name=f"tpb{u}_{k}")
                       for k in range(2)]
                first, last = (u == 0), (u == 7)
                for j in range(4):
                    xl = xn[:, j * 256: j * 256 + 128]
                    xh = xn[:, j * 256 + 128: j * 256 + 256]
                    xf = xn[:, j * 256: j * 256 + 256]
                    nc.tensor.matmul(g0[:], xl, xf,
                                     start=(first and j == 0),
                                     stop=(last and j == 3))
                    nc.tensor.matmul(g1h[:], xh, xh,
                                     start=(first and j == 0),
                                     stop=(last and j == 3))
                for j in range(4):
                    nc.tensor.matmul(tpb[0][:, j * 128:(j + 1) * 128],
                                     xn[:, j * 256: j * 256 + 128],
                                     idb[:], is_transpose=True,
                                     skip_group_check=True)
                    nc.tensor.matmul(tpb[1][:, j * 128:(j + 1) * 128],
                                     xn[:, j * 256 + 128: j * 256 + 256],
                                     idb[:], is_transpose=True,
                                     skip_group_check=True)
                for k in range(2):
                    dst = xt_t[k][:, u * 512:(u + 1) * 512]
                    if (u + k) % 2 == 0:
                        nc.vector.tensor_copy(dst, tpb[k][:])
                    else:
                        nc.scalar.copy(dst, tpb[k][:])

            # ---- phase 2: chain (all bf16)
            # G blocks: block0 = G[0:128,:256] = g0; block1 = [tp(g0[:,128:]) | g1h]
            gs0 = bpool.tile([128, 256], BF16, tag="gs0")
            nc.vector.tensor_copy(gs0[:], g0[:])
            gs1h = bpool.tile([128, 128], BF16, tag="gs1h")
            nc.scalar.copy(gs1h[:], g1h[:])
            gtp = psw.tile([128, 128], BF16, tag="w")
            nc.tensor.matmul(gtp[:], gs0[:, 128:256], idb[:],
                             is_transpose=True, skip_group_check=True)
            for _ in range(3):
                nc.tensor.matmul(wu[:], idz[:, 0:128], idz[:],
                                 start=True, stop=True, skip_group_check=True)
            gs1l = bpool.tile([128, 128], BF16, tag="gs1l")
            nc.vector.tensor_copy(gs1l[:], gtp[:])

            # keep PE warm through the latency-bound chain
            def warm(n):
                for _ in range(n):
                    nc.tensor.matmul(wu[:], idz[:, 0:128], idz[:],
                                     start=True, stop=True,
                                     skip_group_check=True)

            # T = G R  (R = Wg WwS/N host-folded), via symmetry of G
            t_ps = [psw.tile([128, 256], F32, tag="w", name=f"t_ps{m}")
                    for m in range(2)]
            for m in range(2):
                nc.tensor.matmul(t_ps[m][:], gs0[:, m * 128:(m + 1) * 128],
                                 rp[:, 0:256], start=True, stop=False)
                nc.tensor.matmul(t_ps[m][:], (gs1l if m == 0 else gs1h)[:],
                                 rp[:, 256:512], start=False, stop=True)
            warm(3)
            t_s = [bpool.tile([128, 256], BF16, tag=f"t_s{m}", name=f"t_s{m}")
                   for m in range(2)]
            nc.vector.tensor_copy(t_s[0][:], t_ps[0][:])
            nc.scalar.copy(t_s[1][:], t_ps[1][:])

            # M2' = L T + I  (L = Wt Wphi^T host-folded; lhsT slices of L^T)
            m2_s = [bpool.tile([128, 256], BF16, tag=f"m2_{k}", name=f"m2_{k}")
                    for k in range(2)]
            m_ps = [psw.tile([128, 256], F32, tag="w", name=f"m_ps{k}")
                    for k in range(2)]
            for kin in range(2):
                for blk in range(2):
                    nc.tensor.matmul(
                        m_ps[kin][:],
                        lt[:, blk * 256 + kin * 128: blk * 256 + (kin + 1) * 128],
                        t_s[blk][:], start=(blk == 0), stop=(blk == 1))
            warm(4)
            nc.vector.tensor_add(m2_s[0][:], m_ps[0][:], id2[:, 0:256])
            nc.vector.tensor_add(m2_s[1][:], m_ps[1][:], id2[:, 256:512])

            # ---- phase 3: z^T = M2'^T X^T + t2  (stationary M2' slices)
            for m in range(2):
                for gq in range(2):
                    ps_list = []
                    for k in range(2):
                        lhs = m2_s[k][:, m * 128:(m + 1) * 128]
                        for gi in range(4):
                            g = gq * 4 + gi
                            if k == 0:
                                ps = psw.tile([128, 512], F32, tag="w", name=f"zp{m}_{gq}_{gi}")
                                ps_list.append(ps)
                            nc.tensor.matmul(
                                ps_list[gi][:], lhs,
                                xt_t[k][:, g * 512:(g + 1) * 512],
                                start=(k == 0), stop=(k == 1),
                            )
                    for gi in range(4):
                        g = gq * 4 + gi
                        idx = m * 8 + g
                        if gi % 2 == 0:
                            z_s = zpool.tile([128, 1024], BF16, tag="z",
                                             name=f"z_s{idx}")
                        half = (gi % 2) * 512
                        # column order stays permuted; host fixes it up
                        if idx % 2 == 0:
                            nc.vector.tensor_scalar_add(
                                z_s[:, half:half + 512], ps_list[gi][:],
                                t2c[:, m:m + 1])
                        else:
                            nc.scalar.activation(
                                z_s[:, half:half + 512], ps_list[gi][:],
                                mybir.ActivationFunctionType.Identity,
                                bias=t2c[:, m:m + 1])
                        if m == 1 and gq == 1:
                            (nc.sync if gi % 2 == 0 else nc.scalar).dma_start(
                                z_d[m * 128:(m + 1) * 128,
                                    g * 512:(g + 1) * 512],
                                z_s[:, half:half + 512])
                        elif gi % 2 == 1:
                            (nc.sync if (m * 2 + gq) % 2 == 0
                             else nc.scalar).dma_start(
                                z_d[m * 128:(m + 1) * 128,
                                    (g - 1) * 512:(g + 1) * 512], z_s[:])

    nc.compile()
    return nc


def _get_nc():
    if "nc" not in _CACHE:
        _CACHE["nc"] = _build_nc()
    return _CACHE["nc"]


def _fold_params(w_g, b_g, w_theta, b_theta, w_phi, b_phi, w_W, b_W,
                 bn_gamma, bn_beta, bn_mean, bn_var):
    f32, bf16 = np.float32, ml_dtypes.bfloat16
    s = (bn_gamma / np.sqrt(bn_var + BN_EPS)).astype(f32)
    t2 = ((b_W - bn_mean) * s + bn_beta).astype(f32)
    pack = lambda w: np.ascontiguousarray(
        np.concatenate([w[:128, :], w[128:, :]], axis=1), dtype=bf16)
    # M2 = L G R with L = Wt Wphi^T, R = Wg (Ww*s)/N  (all folded on host)
    L = np.asarray(w_theta, f32) @ np.asarray(w_phi, f32).T
    R = np.asarray(w_g, f32) @ (np.asarray(w_W, f32) * s[None, :] / N)
    lt_p = pack(L.T)
    rp_p = pack(R)
    t2c = np.ascontiguousarray(t2.reshape(2, 128).T, dtype=f32)
    return lt_p, rp_p, t2c


def _reference_fallback(x, w_g, b_g, w_theta, b_theta, w_phi, b_phi, w_W, b_W,
                        bn_gamma, bn_beta, bn_mean, bn_var):
    b, h, w, c = x.shape
    n = h * w
    xf = x.reshape(b, n, c).astype(np.float32)
    g_x = xf @ w_g + b_g
    theta_x = xf @ w_theta + b_theta
    phi_x = xf @ w_phi + b_phi
    a = np.einsum("bnd,bne->bde", phi_x, g_x) / n
    y = theta_x @ a
    w_y = y @ w_W + b_W
    w_y = bn_gamma * (w_y - bn_mean) / np.sqrt(bn_var + BN_EPS) + bn_beta
    return (w_y.reshape(b, h, w, c) + x).astype(np.float32)


def run_sharded(x, folded, trace=False):
    from concourse.bass_utils import run_bass_kernel_spmd

    nc = _get_nc()
    lt_p, rp_p, t2c = folded
    bf16 = ml_dtypes.bfloat16
    xr = np.ascontiguousarray(
        np.asarray(x, dtype=np.float32).reshape(B, N, C).astype(bf16))
    idb = np.eye(128, dtype=bf16)
    in_maps = [
        {"x": xr[i], "lt": lt_p, "rp": rp_p, "idb": idb, "t2c": t2c}
        for i in range(B)
    ]
    res = run_bass_kernel_spmd(nc, in_maps, list(range(B)), trace=trace)
    # device column order within each 512-window is f = j*128 + p for
    # spatial n = u*512 + 4p + j; undo that, then transpose [C,N] -> [N,C]
    def fix(zt):
        zt = np.asarray(zt).reshape(C, 8, 4, 128).transpose(0, 1, 3, 2)
        return zt.reshape(C, N).T.astype(np.float32)
    z = np.stack([fix(res.results[i]["zt"]) for i in range(B)], axis=0)
    return z.reshape(B, H, W, C), res


def kernel(x, w_g, b_g, w_theta, b_theta, w_phi, b_phi, w_W, b_W,
           bn_gamma, bn_beta, bn_mean, bn_var):
    args = dict(w_g=np.asarray(w_g), b_g=np.asarray(b_g),
                w_theta=np.asarray(w_theta), b_theta=np.asarray(b_theta),
                w_phi=np.asarray(w_phi), b_phi=np.asarray(b_phi),
                w_W=np.asarray(w_W), b_W=np.asarray(b_W),
                bn_gamma=np.asarray(bn_gamma), bn_beta=np.asarray(bn_beta),
                bn_mean=np.asarray(bn_mean), bn_var=np.asarray(bn_var))
    x = np.asarray(x)
    # the device path folds the (zero) projection biases away; anything else
    # (never produced by setup_inputs) gets the exact host fallback
    if (np.any(args["b_g"]) or np.any(args["b_theta"]) or np.any(args["b_phi"])
            or x.shape != (B, H, W, C)):
        return _reference_fallback(x, **{k: v for k, v in args.items()})
    folded = _fold_params(**args)
    z, _ = run_sharded(x, folded)
    return z


# revision 14
# speedup vs baseline: 1.0090x; 1.0090x over previous
"""Non-local block (no softmax) on 8 Trainium2 cores, data-parallel over batch.

Math: per sample X [N=4096, C=256] (N = 64*64 spatial, C channels):
    theta = X Wt, phi = X Wp, g = X Wg          (biases are zero)
    y = (theta phi^T / N) g  ->  associativity (no softmax):
      y = X Wt A / N,  A = Wp^T G Wg,  G = X^T X
    z = y (Ww * s) + t2 + X  =  X (M2 + I) + t2
      M2 = Wt (A/N) (Ww * s),  s = gamma*rsqrt(var+eps),
      t2 = (b_W - mean)*s + beta
so per core: G = X^T X, a small 256x256 chain to build M2' = M2 + I,
then z^T = M2'^T X^T + t2. One sample per NeuronCore.

v2 layout notes:
  - x rows are loaded permuted: partition p holds rows 4p..4p+3 of each
    512-row block (2KB contiguous per partition per DMA -> efficient
    descriptors). G is row-permutation invariant.
  - X^T windows carry the permuted order f = j*128 + p  <->  n = 4p + j;
    the phase-3 PSUM drain un-permutes via a rearranged read AP so the
    z stores are contiguous.
  - G symmetry: G[128:,0:128] is rebuilt as transpose(G[0:128,128:]).
  - chain runs in bf16; z output is stored bf16 and upcast on host.
"""

import numpy as np
import ml_dtypes

B, H, W, C = 8, 64, 64, 256
IC = C // 2
N = H * W
BN_EPS = 1e-3

_CACHE = {}


def _build_nc():
    import concourse.bacc as bacc
    import concourse.mybir as mybir
    import concourse.tile as tile

    F32 = mybir.dt.float32
    BF16 = mybir.dt.bfloat16

    nc = bacc.Bacc("TRN2", target_bir_lowering=False, debug=False)

    x_d = nc.dram_tensor("x", [N, C], BF16, kind="ExternalInput")
    lt_d = nc.dram_tensor("lt", [128, 512], BF16, kind="ExternalInput")
    rp_d = nc.dram_tensor("rp", [128, 512], BF16, kind="ExternalInput")
    idb_d = nc.dram_tensor("idb", [128, 128], BF16, kind="ExternalInput")
    t2c_d = nc.dram_tensor("t2c", [128, 2], F32, kind="ExternalInput")
    z_d = nc.dram_tensor("zt", [C, N], BF16, kind="ExternalOutput")

    with tile.TileContext(nc) as tc:
        with (
            tc.tile_pool(name="const", bufs=1) as cpool,
            tc.tile_pool(name="big", bufs=1) as bpool,
            tc.tile_pool(name="zs", bufs=5) as zpool,
            tc.tile_pool(name="psg", bufs=1, space="PSUM") as psg,
            tc.tile_pool(name="psw", bufs=6, space="PSUM") as psw,
        ):
            lt = cpool.tile([128, 512], BF16, tag="lt")
            rp = cpool.tile([128, 512], BF16, tag="rp")
            id2 = cpool.tile([128, 512], BF16, tag="id2")
            idb = cpool.tile([128, 128], BF16, tag="idb")
            t2c = cpool.tile([128, 2], F32, tag="t2c")
            idz = cpool.tile([128, 512], BF16, tag="idz")
            nc.vector.memset(idz[:], 0.0)
            # aux tensors ride the gpsimd (SWDGE) ring so the two HWDGE
            # rings carry nothing but x tiles / z stores
            nc.gpsimd.dma_start(idb[:], idb_d[:])
            nc.gpsimd.dma_start(t2c[:], t2c_d[:])
            # id2 = [eye128 | 0 ; 0 | eye128] assembled from idb on-chip
            nc.vector.memset(id2[:], 0.0)
            nc.vector.tensor_copy(id2[:, 0:128], idb[:])
            nc.scalar.copy(id2[:, 384:512], idb[:])

            xnat_t = [bpool.tile([128, 1024], BF16, tag=f"x_nat{u}", name=f"x_nat{u}")
                      for u in range(8)]
            # X^T halves: xt[k][c', u*512 + j*128 + p] = X[u*512 + 4p + j, 128k + c']
            xt_t = [bpool.tile([128, 4096], BF16, tag=f"xt{k}", name=f"xt{k}")
                    for k in range(2)]

            # PE warmup on zeros: keep the clock ramping while x loads
            wu = psw.tile([128, 512], F32, tag="w")
            for _ in range(8):
                nc.tensor.matmul(wu[:, 0:128], idz[:, 0:128], idz[:, 0:128],
                                 start=True, stop=True, skip_group_check=True)

            # ---- phase 1: load x; G = X^T X (sym-reduced); X^T via PE transpose
            for u in range(8):
                # partition p <- rows 4p..4p+3 (2KB contiguous per partition)
                (nc.sync if u % 2 == 0 else nc.scalar).dma_start(
                    xnat_t[u][:].rearrange("p (j c) -> p j c", j=4),
                    x_d[u * 512:(u + 1) * 512, :].rearrange(
                        "(p j) c -> p j c", p=128))

            g0 = psg.tile([128, 256], F32, tag="g0")
            g1h = psg.tile([128, 128], F32, tag="g1h")
            for u in range(8):
                if u == 2:
                    nc.gpsimd.dma_start(lt[:], lt_d[:])
                    nc.gpsimd.dma_start(rp[:], rp_d[:])
                xn = xnat_t[u]
                tpb = [psw.tile([128, 512], BF16, tag="w", name=f"tpb{u}_{k}")
                       for k in range(2)]
                first, last = (u == 0), (u == 7)
                for j in range(4):
                    xl = xn[:, j * 256: j * 256 + 128]
                    xh = xn[:, j * 256 + 128: j * 256 + 256]
                    xf = xn[:, j * 256: j * 256 + 256]
                    # lhsT = xl for both the G row-block and its transpose
                    nc.tensor.matmul(g0[:], xl, xf,
                                     start=(first and j == 0),
                                     stop=(last and j == 3))
                    nc.tensor.matmul(tpb[0][:, j * 128:(j + 1) * 128],
                                     xl, idb[:], is_transpose=True,
                                     skip_group_check=True)
                    nc.tensor.matmul(g1h[:], xh, xh,
                                     start=(first and j == 0),
                                     stop=(last and j == 3))
                    nc.tensor.matmul(tpb[1][:, j * 128:(j + 1) * 128],
                                     xh, idb[:], is_transpose=True,
                                     skip_group_check=True)
                for k in range(2):
                    dst = xt_t[k][:, u * 512:(u + 1) * 512]
                    if (u + k) % 2 == 0:
                        nc.vector.tensor_copy(dst, tpb[k][:])
                    else:
                        nc.scalar.copy(dst, tpb[k][:])

            # ---- phase 2: chain (all bf16)
            # G blocks: block0 = G[0:128,:256] = g0; block1 = [tp(g0[:,128:]) | g1h]
            gs0 = bpool.tile([128, 256], BF16, tag="gs0")
            nc.vector.tensor_copy(gs0[:], g0[:])
            gs1h = bpool.tile([128, 128], BF16, tag="gs1h")
            nc.scalar.copy(gs1h[:], g1h[:])
            gtp = psw.tile([128, 128], BF16, tag="w")
            nc.tensor.matmul(gtp[:], gs0[:, 128:256], idb[:],
                             is_transpose=True, skip_group_check=True)
            for _ in range(3):
                nc.tensor.matmul(wu[:], idz[:, 0:128], idz[:],
                                 start=True, stop=True, skip_group_check=True)
            gs1l = bpool.tile([128, 128], BF16, tag="gs1l")
            nc.vector.tensor_copy(gs1l[:], gtp[:])

            # keep PE warm through the latency-bound chain
            def warm(n):
                for _ in range(n):
                    nc.tensor.matmul(wu[:], idz[:, 0:128], idz[:],
                                     start=True, stop=True,
                                     skip_group_check=True)

            # T = G R  (R = Wg WwS/N host-folded), via symmetry of G
            t_ps = [psw.tile([128, 256], F32, tag="w", name=f"t_ps{m}")
                    for m in range(2)]
            for m in range(2):
                nc.tensor.matmul(t_ps[m][:], gs0[:, m * 128:(m + 1) * 128],
                                 rp[:, 0:256], start=True, stop=False)
                nc.tensor.matmul(t_ps[m][:], (gs1l if m == 0 else gs1h)[:],
                                 rp[:, 256:512], start=False, stop=True)
            warm(3)
            t_s = [bpool.tile([128, 256], BF16, tag=f"t_s{m}", name=f"t_s{m}")
                   for m in range(2)]
            nc.vector.tensor_copy(t_s[0][:], t_ps[0][:])
            nc.scalar.copy(t_s[1][:], t_ps[1][:])

            # M2' = L T + I  (L = Wt Wphi^T host-folded; lhsT slices of L^T)
            m2_s = [bpool.tile([128, 256], BF16, tag=f"m2_{k}", name=f"m2_{k}")
                    for k in range(2)]
            m_ps = [psw.tile([128, 256], F32, tag="w", name=f"m_ps{k}")
                    for k in range(2)]
            for kin in range(2):
                for blk in range(2):
                    nc.tensor.matmul(
                        m_ps[kin][:],
                        lt[:, blk * 256 + kin * 128: blk * 256 + (kin + 1) * 128],
                        t_s[blk][:], start=(blk == 0), stop=(blk == 1))
            warm(4)
            nc.vector.tensor_add(m2_s[0][:], m_ps[0][:], id2[:, 0:256])
            nc.vector.tensor_add(m2_s[1][:], m_ps[1][:], id2[:, 256:512])

            # ---- phase 3: z^T = M2'^T X^T + t2  (stationary M2' slices)
            for m in range(2):
                for gq in range(2):
                    ps_list = []
                    for k in range(2):
                        lhs = m2_s[k][:, m * 128:(m + 1) * 128]
                        for gi in range(4):
                            g = gq * 4 + gi
                            if k == 0:
                                ps = psw.tile([128, 512], F32, tag="w", name=f"zp{m}_{gq}_{gi}")
                                ps_list.append(ps)
                            nc.tensor.matmul(
                                ps_list[gi][:], lhs,
                                xt_t[k][:, g * 512:(g + 1) * 512],
                                start=(k == 0), stop=(k == 1),
                            )
                    for gi in range(4):
                        g = gq * 4 + gi
                        idx = m * 8 + g
                        if gi % 2 == 0:
                            z_s = zpool.tile([128, 1024], BF16, tag="z",
                                             name=f"z_s{idx}")
                        half = (gi % 2) * 512
                        # column order stays permuted; host fixes it up
                        if idx % 2 == 0:
                            nc.vector.tensor_scalar_add(
                                z_s[:, half:half + 512], ps_list[gi][:],
                                t2c[:, m:m + 1])
                        else:
                            nc.scalar.activation(
                                z_s[:, half:half + 512], ps_list[gi][:],
                                mybir.ActivationFunctionType.Identity,
                                bias=t2c[:, m:m + 1])
                        if m == 1 and gq == 1:
                            if gi < 3:
                                (nc.sync if gi % 2 == 0
                                 else nc.scalar).dma_start(
                                    z_d[m * 128:(m + 1) * 128,
                                        g * 512:(g + 1) * 512],
                                    z_s[:, half:half + 512])
                            else:
                                nc.scalar.dma_start(
                                    z_d[m * 128:(m + 1) * 128,
                                        g * 512:g * 512 + 384],
                                    z_s[:, half:half + 384])
                                nc.sync.dma_start(
                                    z_d[m * 128:(m + 1) * 128,
                                        g * 512 + 384:(g + 1) * 512],
                                    z_s[:, half + 384:half + 512])
                        elif gi % 2 == 1:
                            (nc.sync if (m * 2 + gq) % 2 == 0
                             else nc.scalar).dma_start(
                                z_d[m * 128:(m + 1) * 128,
                                    (g - 1) * 512:(g + 1) * 512], z_s[:])

    nc.compile()
    return nc


def _get_nc():
    if "nc" not in _CACHE:
        _CACHE["nc"] = _build_nc()
    return _CACHE["nc"]


def _fold_params(w_g, b_g, w_theta, b_theta, w_phi, b_phi, w_W, b_W,
                 bn_gamma, bn_beta, bn_mean, bn_var):
    f32, bf16 = np.float32, ml_dtypes.bfloat16
    s = (bn_gamma / np.sqrt(bn_var + BN_EPS)).astype(f32)
    t2 = ((b_W - bn_mean) * s + bn_beta).astype(f32)
    pack = lambda w: np.ascontiguousarray(
        np.concatenate([w[:128, :], w[128:, :]], axis=1), dtype=bf16)
    # M2 = L G R with L = Wt Wphi^T, R = Wg (Ww*s)/N  (all folded on host)
    L = np.asarray(w_theta, f32) @ np.asarray(w_phi, f32).T
    R = np.asarray(w_g, f32) @ (np.asarray(w_W, f32) * s[None, :] / N)
    lt_p = pack(L.T)
    rp_p = pack(R)
    t2c = np.ascontiguousarray(t2.reshape(2, 128).T, dtype=f32)
    return lt_p, rp_p, t2c


def _reference_fallback(x, w_g, b_g, w_theta, b_theta, w_phi, b_phi, w_W, b_W,
                        bn_gamma, bn_beta, bn_mean, bn_var):
    b, h, w, c = x.shape
    n = h * w
    xf = x.reshape(b, n, c).astype(np.float32)
    g_x = xf @ w_g + b_g
    theta_x = xf @ w_theta + b_theta
    phi_x = xf @ w_phi + b_phi
    a = np.einsum("bnd,bne->bde", phi_x, g_x) / n
    y = theta_x @ a
    w_y = y @ w_W + b_W
    w_y = bn_gamma * (w_y - bn_mean) / np.sqrt(bn_var + BN_EPS) + bn_beta
    return (w_y.reshape(b, h, w, c) + x).astype(np.float32)


def run_sharded(x, folded, trace=False):
    from concourse.bass_utils import run_bass_kernel_spmd

    nc = _get_nc()
    lt_p, rp_p, t2c = folded
    bf16 = ml_dtypes.bfloat16
    xr = np.ascontiguousarray(
        np.asarray(x, dtype=np.float32).reshape(B, N, C).astype(bf16))
    idb = np.eye(128, dtype=bf16)
    in_maps = [
        {"x": xr[i], "lt": lt_p, "rp": rp_p, "idb": idb, "t2c": t2c}
        for i in range(B)
    ]
    res = run_bass_kernel_spmd(nc, in_maps, list(range(B)), trace=trace)
    # device column order within each 512-window is f = j*128 + p for
    # spatial n = u*512 + 4p + j; undo that, then transpose [C,N] -> [N,C]
    def fix(zt):
        zt = np.asarray(zt).reshape(C, 8, 4, 128).transpose(0, 1, 3, 2)
        return zt.reshape(C, N).T.astype(np.float32)
    z = np.stack([fix(res.results[i]["zt"]) for i in range(B)], axis=0)
    return z.reshape(B, H, W, C), res


def kernel(x, w_g, b_g, w_theta, b_theta, w_phi, b_phi, w_W, b_W,
           bn_gamma, bn_beta, bn_mean, bn_var):
    args = dict(w_g=np.asarray(w_g), b_g=np.asarray(b_g),
                w_theta=np.asarray(w_theta), b_theta=np.asarray(b_theta),
                w_phi=np.asarray(w_phi), b_phi=np.asarray(b_phi),
                w_W=np.asarray(w_W), b_W=np.asarray(b_W),
                bn_gamma=np.asarray(bn_gamma), bn_beta=np.asarray(bn_beta),
                bn_mean=np.asarray(bn_mean), bn_var=np.asarray(bn_var))
    x = np.asarray(x)
    # the device path folds the (zero) projection biases away; anything else
    # (never produced by setup_inputs) gets the exact host fallback
    if (np.any(args["b_g"]) or np.any(args["b_theta"]) or np.any(args["b_phi"])
            or x.shape != (B, H, W, C)):
        return _reference_fallback(x, **{k: v for k, v in args.items()})
    folded = _fold_params(**args)
    z, _ = run_sharded(x, folded)
    return z


# revision 15
# speedup vs baseline: 1.0513x; 1.0419x over previous
"""Non-local block (no softmax) on 8 Trainium2 cores, data-parallel over batch.

Math: per sample X [N=4096, C=256] (N = 64*64 spatial, C channels):
    theta = X Wt, phi = X Wp, g = X Wg          (biases are zero)
    y = (theta phi^T / N) g  ->  associativity (no softmax):
      y = X Wt A / N,  A = Wp^T G Wg,  G = X^T X
    z = y (Ww * s) + t2 + X  =  X (M2 + I) + t2
      M2 = Wt (A/N) (Ww * s),  s = gamma*rsqrt(var+eps),
      t2 = (b_W - mean)*s + beta
so per core: G = X^T X, a small 256x256 chain to build M2' = M2 + I,
then z^T = M2'^T X^T + t2. One sample per NeuronCore.

v2 layout notes:
  - x rows are loaded permuted: partition p holds rows 4p..4p+3 of each
    512-row block (2KB contiguous per partition per DMA -> efficient
    descriptors). G is row-permutation invariant.
  - X^T windows carry the permuted order f = j*128 + p  <->  n = 4p + j;
    the phase-3 PSUM drain un-permutes via a rearranged read AP so the
    z stores are contiguous.
  - G symmetry: G[128:,0:128] is rebuilt as transpose(G[0:128,128:]).
  - chain runs in bf16; z output is stored bf16 and upcast on host.
"""

import numpy as np
import ml_dtypes

B, H, W, C = 8, 64, 64, 256
IC = C // 2
N = H * W
BN_EPS = 1e-3

_CACHE = {}


def _build_nc():
    import concourse.bacc as bacc
    import concourse.mybir as mybir
    import concourse.tile as tile

    F32 = mybir.dt.float32
    BF16 = mybir.dt.bfloat16

    nc = bacc.Bacc("TRN2", target_bir_lowering=False, debug=False)

    x_d = nc.dram_tensor("x", [N, C], BF16, kind="ExternalInput")
    lt_d = nc.dram_tensor("lt", [128, 512], BF16, kind="ExternalInput")
    rp_d = nc.dram_tensor("rp", [128, 512], BF16, kind="ExternalInput")
    idb_d = nc.dram_tensor("idb", [128, 128], BF16, kind="ExternalInput")
    t2c_d = nc.dram_tensor("t2c", [128, 2], F32, kind="ExternalInput")
    z_d = nc.dram_tensor("zt", [C, N], BF16, kind="ExternalOutput")

    with tile.TileContext(nc) as tc:
        with (
            tc.tile_pool(name="const", bufs=1) as cpool,
            tc.tile_pool(name="big", bufs=1) as bpool,
            tc.tile_pool(name="zs", bufs=5) as zpool,
            tc.tile_pool(name="psg", bufs=1, space="PSUM") as psg,
            tc.tile_pool(name="psw", bufs=6, space="PSUM") as psw,
        ):
            lt = cpool.tile([128, 512], BF16, tag="lt")
            rp = cpool.tile([128, 512], BF16, tag="rp")
            id2 = cpool.tile([128, 512], BF16, tag="id2")
            idb = cpool.tile([128, 128], BF16, tag="idb")
            t2c = cpool.tile([128, 2], F32, tag="t2c")
            idz = cpool.tile([128, 512], BF16, tag="idz")
            nc.vector.memset(idz[:], 0.0)
            # aux tensors ride the gpsimd (SWDGE) ring so the two HWDGE
            # rings carry nothing but x tiles / z stores
            nc.gpsimd.dma_start(idb[:], idb_d[:])
            nc.gpsimd.dma_start(t2c[:], t2c_d[:])
            # id2 = [eye128 | 0 ; 0 | eye128] assembled from idb on-chip
            nc.vector.memset(id2[:], 0.0)
            nc.vector.tensor_copy(id2[:, 0:128], idb[:])
            nc.scalar.copy(id2[:, 384:512], idb[:])

            xnat_t = [bpool.tile([128, 1024], BF16, tag=f"x_nat{u}", name=f"x_nat{u}")
                      for u in range(8)]
            # X^T halves: xt[k][c', u*512 + j*128 + p] = X[u*512 + 4p + j, 128k + c']
            xt_t = [bpool.tile([128, 4096], BF16, tag=f"xt{k}", name=f"xt{k}")
                    for k in range(2)]

            # PE warmup on zeros: keep the clock ramping while x loads
            wu = psw.tile([128, 512], F32, tag="w")
            for _ in range(8):
                nc.tensor.matmul(wu[:, 0:128], idz[:, 0:128], idz[:, 0:128],
                                 start=True, stop=True, skip_group_check=True)

            # ---- phase 1: load x; G = X^T X (sym-reduced); X^T via PE transpose
            for u in range(8):
                # partition p <- rows 4p..4p+3 (2KB contiguous per partition)
                (nc.sync if u % 2 == 0 else nc.scalar).dma_start(
                    xnat_t[u][:].rearrange("p (j c) -> p j c", j=4),
                    x_d[u * 512:(u + 1) * 512, :].rearrange(
                        "(p j) c -> p j c", p=128))

            g0 = psg.tile([128, 256], F32, tag="g0")
            g1h = psg.tile([128, 128], F32, tag="g1h")
            for u in range(8):
                if u == 2:
                    nc.gpsimd.dma_start(lt[:], lt_d[:])
                    nc.gpsimd.dma_start(rp[:], rp_d[:])
                xn = xnat_t[u]
                tpb = [psw.tile([128, 512], BF16, tag="w", name=f"tpb{u}_{k}")
                       for k in range(2)]
                first, last = (u == 0), (u == 7)
                for j in range(4):
                    xl = xn[:, j * 256: j * 256 + 128]
                    xh = xn[:, j * 256 + 128: j * 256 + 256]
                    xf = xn[:, j * 256: j * 256 + 256]
                    # lhsT = xl for both the G row-block and its transpose
                    nc.tensor.matmul(g0[:], xl, xf,
                                     start=(first and j == 0),
                                     stop=(last and j == 3))
                    nc.tensor.matmul(tpb[0][:, j * 128:(j + 1) * 128],
                                     xl, idb[:], is_transpose=True,
                                     skip_group_check=True)
                    nc.tensor.matmul(g1h[:], xh, xh,
                                     start=(first and j == 0),
                                     stop=(last and j == 3))
                    nc.tensor.matmul(tpb[1][:, j * 128:(j + 1) * 128],
                                     xh, idb[:], is_transpose=True,
                                     skip_group_check=True)
                for k in range(2):
                    dst = xt_t[k][:, u * 512:(u + 1) * 512]
                    if (u + k) % 2 == 0:
                        nc.vector.tensor_copy(dst, tpb[k][:])
                    else:
                        nc.scalar.copy(dst, tpb[k][:])

            # ---- phase 2: chain (all bf16)
            # G blocks: block0 = G[0:128,:256] = g0; block1 = [tp(g0[:,128:]) | g1h]
            gs0 = bpool.tile([128, 256], BF16, tag="gs0")
            nc.vector.tensor_copy(gs0[:], g0[:])
            gs1h = bpool.tile([128, 128], BF16, tag="gs1h")
            nc.scalar.copy(gs1h[:], g1h[:])
            gtp = psw.tile([128, 128], BF16, tag="w")
            nc.tensor.matmul(gtp[:], gs0[:, 128:256], idb[:],
                             is_transpose=True, skip_group_check=True)
            for _ in range(3):
                nc.tensor.matmul(wu[:], idz[:, 0:128], idz[:],
                                 start=True, stop=True, skip_group_check=True)
            gs1l = bpool.tile([128, 128], BF16, tag="gs1l")
            nc.vector.tensor_copy(gs1l[:], gtp[:])

            # keep PE warm through the latency-bound chain
            def warm(n):
                for _ in range(n):
                    nc.tensor.matmul(wu[:], idz[:, 0:128], idz[:],
                                     start=True, stop=True,
                                     skip_group_check=True)

            # T = G R  (R = Wg WwS/N host-folded), via symmetry of G
            t_ps = [psw.tile([128, 256], F32, tag="w", name=f"t_ps{m}")
                    for m in range(2)]
            for m in range(2):
                nc.tensor.matmul(t_ps[m][:], gs0[:, m * 128:(m + 1) * 128],
                                 rp[:, 0:256], start=True, stop=False)
                nc.tensor.matmul(t_ps[m][:], (gs1l if m == 0 else gs1h)[:],
                                 rp[:, 256:512], start=False, stop=True)
            warm(3)
            t_s = [bpool.tile([128, 256], BF16, tag=f"t_s{m}", name=f"t_s{m}")
                   for m in range(2)]
            nc.vector.tensor_copy(t_s[0][:], t_ps[0][:])
            nc.scalar.copy(t_s[1][:], t_ps[1][:])

            # M2' = L T + I  (L = Wt Wphi^T host-folded; lhsT slices of L^T)
            m2_s = [bpool.tile([128, 256], BF16, tag=f"m2_{k}", name=f"m2_{k}")
                    for k in range(2)]
            m_ps = [psw.tile([128, 256], F32, tag="w", name=f"m_ps{k}")
                    for k in range(2)]
            for kin in range(2):
                for blk in range(2):
                    nc.tensor.matmul(
                        m_ps[kin][:],
                        lt[:, blk * 256 + kin * 128: blk * 256 + (kin + 1) * 128],
                        t_s[blk][:], start=(blk == 0), stop=(blk == 1))
            warm(4)
            nc.vector.tensor_add(m2_s[0][:], m_ps[0][:], id2[:, 0:256])
            nc.vector.tensor_add(m2_s[1][:], m_ps[1][:], id2[:, 256:512])

            # ---- phase 3: z^T = M2'^T X^T + t2  (stationary M2' slices)
            for m in range(2):
                for gq in range(2):
                    ps_list = []
                    for k in range(2):
                        lhs = m2_s[k][:, m * 128:(m + 1) * 128]
                        for gi in range(4):
                            g = gq * 4 + gi
                            if k == 0:
                                ps = psw.tile([128, 512], F32, tag="w", name=f"zp{m}_{gq}_{gi}")
                                ps_list.append(ps)
                            nc.tensor.matmul(
                                ps_list[gi][:], lhs,
                                xt_t[k][:, g * 512:(g + 1) * 512],
                                start=(k == 0), stop=(k == 1),
                            )
                    for gi in range(4):
                        g = gq * 4 + gi
                        idx = m * 8 + g
                        if gi % 2 == 0:
                            z_s = zpool.tile([128, 1024], BF16, tag="z",
                                             name=f"z_s{idx}")
                        half = (gi % 2) * 512
                        # column order stays permuted; host fixes it up
                        if idx % 2 == 0:
                            nc.vector.tensor_scalar_add(
                                z_s[:, half:half + 512], ps_list[gi][:],
                                t2c[:, m:m + 1])
                        else:
                            nc.scalar.activation(
                                z_s[:, half:half + 512], ps_list[gi][:],
                                mybir.ActivationFunctionType.Identity,
                                bias=t2c[:, m:m + 1])
                        if m == 1 and gq == 1:
                            (nc.sync if gi % 2 == 0 else nc.scalar).dma_start(
                                z_d[m * 128:(m + 1) * 128,
                                    g * 512:(g + 1) * 512],
                                z_s[:, half:half + 512])
                        elif gi % 2 == 1:
                            (nc.sync if (m * 2 + gq) % 2 == 0
                             else nc.scalar).dma_start(
                                z_d[m * 128:(m + 1) * 128,
                                    (g - 1) * 512:(g + 1) * 512], z_s[:])

    nc.compile()
    return nc


def _get_nc():
    if "nc" not in _CACHE:
        _CACHE["nc"] = _build_nc()
    return _CACHE["nc"]


def _fold_params(w_g, b_g, w_theta, b_theta, w_phi, b_phi, w_W, b_W,
                 bn_gamma, bn_beta, bn_mean, bn_var):
    f32, bf16 = np.float32, ml_dtypes.bfloat16
    s = (bn_gamma / np.sqrt(bn_var + BN_EPS)).astype(f32)
    t2 = ((b_W - bn_mean) * s + bn_beta).astype(f32)
    pack = lambda w: np.ascontiguousarray(
        np.concatenate([w[:128, :], w[128:, :]], axis=1), dtype=bf16)
    # M2 = L G R with L = Wt Wphi^T, R = Wg (Ww*s)/N  (all folded on host)
    L = np.asarray(w_theta, f32) @ np.asarray(w_phi, f32).T
    R = np.asarray(w_g, f32) @ (np.asarray(w_W, f32) * s[None, :] / N)
    lt_p = pack(L.T)
    rp_p = pack(R)
    t2c = np.ascontiguousarray(t2.reshape(2, 128).T, dtype=f32)
    return lt_p, rp_p, t2c


def _reference_fallback(x, w_g, b_g, w_theta, b_theta, w_phi, b_phi, w_W, b_W,
                        bn_gamma, bn_beta, bn_mean, bn_var):
    b, h, w, c = x.shape
    n = h * w
    xf = x.reshape(b, n, c).astype(np.float32)
    g_x = xf @ w_g + b_g
    theta_x = xf @ w_theta + b_theta
    phi_x = xf @ w_phi + b_phi
    a = np.einsum("bnd,bne->bde", phi_x, g_x) / n
    y = theta_x @ a
    w_y = y @ w_W + b_W
    w_y = bn_gamma * (w_y - bn_mean) / np.sqrt(bn_var + BN_EPS) + bn_beta
    return (w_y.reshape(b, h, w, c) + x).astype(np.float32)


def run_sharded(x, folded, trace=False):
    from concourse.bass_utils import run_bass_kernel_spmd

    nc = _get_nc()
    lt_p, rp_p, t2c = folded
    bf16 = ml_dtypes.bfloat16
    xr = np.ascontiguousarray(
        np.asarray(x, dtype=np.float32).reshape(B, N, C).astype(bf16))
    idb = np.eye(128, dtype=bf16)
    in_maps = [
        {"x": xr[i], "lt": lt_p, "rp": rp_p, "idb": idb, "t2c": t2c}
        for i in range(B)
    ]
    res = run_bass_kernel_spmd(nc, in_maps, list(range(B)), trace=trace)
    # device column order within each 512-window is f = j*128 + p for
    # spatial n = u*512 + 4p + j; undo that, then transpose [C,N] -> [N,C]
    def fix(zt):
        zt = np.asarray(zt).reshape(C, 8, 4, 128).transpose(0, 1, 3, 2)
        return zt.reshape(C, N).T.astype(np.float32)
    z = np.stack([fix(res.results[i]["zt"]) for i in range(B)], axis=0)
    return z.reshape(B, H, W, C), res


def kernel(x, w_g, b_g, w_theta, b_theta, w_phi, b_phi, w_W, b_W,
           bn_gamma, bn_beta, bn_mean, bn_var):
    args = dict(w_g=np.asarray(w_g), b_g=np.asarray(b_g),
                w_theta=np.asarray(w_theta), b_theta=np.asarray(b_theta),
                w_phi=np.asarray(w_phi), b_phi=np.asarray(b_phi),
                w_W=np.asarray(w_W), b_W=np.asarray(b_W),
                bn_gamma=np.asarray(bn_gamma), bn_beta=np.asarray(bn_beta),
                bn_mean=np.asarray(bn_mean), bn_var=np.asarray(bn_var))
    x = np.asarray(x)
    # the device path folds the (zero) projection biases away; anything else
    # (never produced by setup_inputs) gets the exact host fallback
    if (np.any(args["b_g"]) or np.any(args["b_theta"]) or np.any(args["b_phi"])
            or x.shape != (B, H, W, C)):
        return _reference_fallback(x, **{k: v for k, v in args.items()})
    folded = _fold_params(**args)
    z, _ = run_sharded(x, folded)
    return z
